# revision 57
# baseline (speedup 1.0000x reference)
"""Trainium2 Bass kernel: multi-head attention with relative-position bias.

Problem shapes: x [8, 1024, 768], H=12 heads, d=64.
Strategy: data-parallel over batch (1 element per NeuronCore, 8 cores).
All matmuls in bf16 (f32 PSUM accumulation). Host prep:
  - weights transposed to [C, *] feature-major; q-scale folded into Wq/q_bias
  - relative-position bias gather done as exp(table)[idx] -> bf16 tensor
    [H, Nj, Ni] streamed from HBM and folded into softmax multiplicatively:
    softmax(s + b) = norm(exp(s) * exp(b))   (no row-max needed: |s| < ~10)
Attention computed transposed (sT[j, i]) so softmax sums run along the PE
contraction: the PV matmul uses stationary [v | 1], giving the denominator as
an extra psum row for free.

v2 restructure (from perfetto trace of v1 @246us):
  - warm-up matmuls at t=0 so the PE HAM clock-gate opens before real work
  - xt/wqkv DMAs issued first (sync DIRECT2D issue is ~0.6us each and was
    serializing startup); wproj/pbias deferred past phase-B emission
  - eb bias tiles batched 2-heads-per-DMA (half the DIRECT2D issues)
  - phase A (qkv projection) almost fully interleaved into phase B's
    scalar-engine-bound stretches: only q0/k0 tiles emitted up front
  - qs PSUM granularity [128,512]x4 banks with per-half exp/mul for finer
    ACT/PE pipelining (kills head-pair boundary stalls -> fewer HAM
    re-throttles)
  - normalization DMA dance batched: one ustage tile per head pair, single
    denominator row DMA [1,2048], one broadcast back [64,2048]
"""
import sys
import numpy as np

sys.path.insert(0, "/opt/trn_rl_repo")

import ml_dtypes

BF16 = ml_dtypes.bfloat16

B, N, C = 8, 1024, 768
H, D = 12, 64
N_CORES = 8
NT = N // 128        # 8 token tiles
CT = C // 128        # 6 feature tiles
OT = 3 * C // 128    # 18 qkv output feature tiles
PV_LAG = 8           # in (t, jc, x) x-steps: 8 = 4 jc iterations

_cache = {}


def _install_axon_shim():
    """The image's antenv lacks axon_hooks; register the NTFF profile hook so
    run_bass_kernel_spmd(trace=True) works. Safe no-op outside axon."""
    import types

    if "antenv.axon_hooks" not in sys.modules:
        try:
            import antenv
            from trn_agent_boot.trn_boot import _ntff_profile_via_ctypes
        except ImportError:
            return
        mod = types.ModuleType("antenv.axon_hooks")
        _hook = [None]
        mod.set_axon_ntff_profile_hook = lambda h: _hook.__setitem__(0, h)
        mod.get_axon_ntff_profile_hook = lambda: _hook[0]
        sys.modules["antenv.axon_hooks"] = mod
        antenv.axon_hooks = mod
        try:
            mod.set_axon_ntff_profile_hook(
                _ntff_profile_via_ctypes("/opt/axon/libaxon_pjrt.so")
            )
        except Exception:
            pass
    from concourse import bass_utils

    bass_utils.upload_artifacts = lambda tmpdir: tmpdir


def build_nc():
    from concourse import bacc, mybir, tile

    f32 = mybir.dt.float32
    bf16 = mybir.dt.bfloat16
    AF = mybir.ActivationFunctionType

    nc = bacc.Bacc("TRN2", target_bir_lowering=False, debug=False,
                   num_devices=N_CORES)

    xt_d = nc.dram_tensor("xt", [C, N], bf16, kind="ExternalInput")
    wqkvt_d = nc.dram_tensor("wqkvt", [C, 3 * C], bf16, kind="ExternalInput")
    qkvb_d = nc.dram_tensor("qkvb", [3 * C], f32, kind="ExternalInput")
    vb_d = nc.dram_tensor("vb", [C], f32, kind="ExternalInput")
    wprojt_d = nc.dram_tensor("wprojt", [C, C], bf16, kind="ExternalInput")
    pbias_d = nc.dram_tensor("pbias", [C], f32, kind="ExternalInput")
    ebt_d = nc.dram_tensor("ebt", [H, N, N], bf16, kind="ExternalInput")
    out_d = nc.dram_tensor("out", [N, C], bf16, kind="ExternalOutput")

    with tile.TileContext(nc) as tc:
        with (
            tc.tile_pool(name="persist", bufs=1) as persist,
            tc.tile_pool(name="work", bufs=1) as work,
            tc.tile_pool(name="dram", bufs=2, space="DRAM") as dpool,
            tc.tile_pool(name="psum", bufs=1, space="PSUM") as psum,
        ):
            # ---- resident tiles ----
            # q feature-major [head_pair][128, N]
            q_sb = [persist.tile([128, N], bf16, tag=f"q{i}", name=f"q{i}") for i in range(6)]
            # k zero-padded per head: [head_pair][128, 2N]; cols 0:N hold head
            # A's k in partitions 0:64 (64:128 zero), cols N:2N hold head B's
            # k in partitions 64:128 (0:64 zero).  K=128 stationaries keep
            # every matmul in 128x128 tiling mode: a 64-row stationary flips
            # the PE into 2x-row-tiled mode and each mode switch costs a
            # ~110ns pipeline drain on the next matmul.
            k_pad = [persist.tile([128, 2 * N], bf16, tag=f"kp{i}", name=f"kp{i}") for i in range(6)]
            # v token-major, 12 groups of (64 vals + 1 one) per token tile
            v_sb = [persist.tile([128, H * 65], bf16, tag=f"v{i}", name=f"v{i}") for i in range(NT)]
            # attention output (pre-proj), feature-major
            ot_sb = [persist.tile([128, N], bf16, tag=f"ot{i}", name=f"ot{i}") for i in range(CT)]

            # ---- PE warm-up: open the HAM clock gate while input DMAs run.
            warm = work.tile([128, 512], bf16, tag="warm")
            nc.vector.memset(warm[:], 0.0)
            for i in range(22):
                wps = psum.tile([128, 512], f32, tag="big", name="wps", bufs=4)
                nc.tensor.matmul(wps[:], warm[:, 0:128], warm[:],
                                 start=True, stop=True, skip_group_check=True)
            # k_pad zero regions (once; after the warm-up emission so the
            # warm-tile memset stays first in the vector-engine queue)
            for t in range(6):
                nc.vector.memset(k_pad[t][64:128, 0:N], 0.0)
                nc.vector.memset(k_pad[t][0:64, N:2 * N], 0.0)

            # ---- input DMAs: x and qkv weights first (per-ct so phase A can
            # start on the first pair), then small constants.
            xt_sb = [work.tile([128, N], bf16, tag=f"xt{i}", name=f"xt{i}") for i in range(CT)]
            wqkv_sb = [work.tile([128, 3 * C], bf16, tag=f"wq{i}", name=f"wq{i}") for i in range(CT)]
            for ct in range(CT):
                nc.sync.dma_start(xt_sb[ct][:], xt_d.ap()[ct * 128:(ct + 1) * 128, :])
                nc.sync.dma_start(wqkv_sb[ct][:], wqkvt_d.ap()[ct * 128:(ct + 1) * 128, :])

            qkvb_sb = persist.tile([128, OT], f32, tag="qkvb")
            vb_bc = persist.tile([128, C], f32, tag="vb_bc")
            nc.sync.dma_start(qkvb_sb[:], qkvb_d.ap().rearrange("(t p) -> p t", p=128))
            nc.sync.dma_start(vb_bc[:], vb_d.ap().unsqueeze(0).to_broadcast([128, C]))

            # wproj/pbias are deferred: emitted mid phase-B (only needed by C)
            wproj_sb = persist.tile([128, CT * C], bf16, tag="wp", name="wp")
            pb_bc = persist.tile([128, C], f32, tag="pb_bc")

            # ================= Phase A pieces =================
            # q,k feature-major: qkvT[o, n] = sum_c WT[c, o] * xT[c, n]
            def emit_qk_tile(ot):
                for h2 in range(2):
                    ps = psum.tile([128, 512], f32, tag="big", name="psa", bufs=4)
                    for ct in range(CT):
                        nc.tensor.matmul(
                            ps[:],
                            wqkv_sb[ct][:, ot * 128:(ot + 1) * 128],
                            xt_sb[ct][:, h2 * 512:(h2 + 1) * 512],
                            start=(ct == 0), stop=(ct == CT - 1),
                            skip_group_check=True,
                        )
                    if ot < 6:
                        nc.vector.tensor_scalar_add(
                            q_sb[ot][:, h2 * 512:(h2 + 1) * 512], ps[:],
                            qkvb_sb[:, ot:ot + 1])
                    else:
                        # k bias is zero; cast psum halves into padded layout
                        t = ot - 6
                        nc.vector.tensor_copy(
                            k_pad[t][0:64, h2 * 512:(h2 + 1) * 512],
                            ps[0:64, :])
                        nc.vector.tensor_copy(
                            k_pad[t][64:128, N + h2 * 512:N + (h2 + 1) * 512],
                            ps[64:128, :])

            # v token-major: v[n, vd] = sum_c xT[c, n] * WT[c, 2C+vd]
            # NOTE: a matmul output may not cross a 2KB PSUM bank, so the
            # 384-wide outputs go to separate [128,512] bank tiles.
            def emit_v_tile(nt):
                pss = []
                for g2 in range(2):
                    ps = psum.tile([128, 512], f32, tag="big", name="psv", bufs=4)
                    pss.append(ps)
                    for ct in range(CT):
                        nc.tensor.matmul(
                            ps[:, 0:384],
                            xt_sb[ct][:, nt * 128:(nt + 1) * 128],
                            wqkv_sb[ct][:, 2 * C + g2 * 384:2 * C + (g2 + 1) * 384],
                            start=(ct == 0), stop=(ct == CT - 1),
                            skip_group_check=True,
                        )
                v_view = v_sb[nt][:].rearrange("p (g c) -> p g c", c=65)
                for g2 in range(2):
                    nc.vector.tensor_add(
                        v_view[:, 6 * g2:6 * (g2 + 1), 0:64],
                        pss[g2][:, 0:384].rearrange("p (h c) -> p h c", c=64),
                        vb_bc[:, g2 * 384:(g2 + 1) * 384]
                        .rearrange("p (h c) -> p h c", c=64),
                    )
                nc.vector.memset(v_view[:, :, 64:65], 1.0)

            emit_qk_tile(0)
            emit_qk_tile(6)

            # deferred phase-A work, drained into phase B's slots
            # x1-slots (after head x=1 of a jc): v tiles in t=0
            # x0-slots: qk tiles, 2 per t starting at t=0 jc 6/7
            slot_x1 = {(0, jc): ("v", jc) for jc in range(NT)}
            slot_x0 = {}
            qk_items = []
            for t in range(1, 6):
                qk_items.append(t)
                qk_items.append(6 + t)
            slot_x0[(0, 5)] = ("qk", qk_items[0])
            slot_x0[(0, 6)] = ("qk", qk_items[1])
            qi = 2
            for t in range(1, 5):
                slot_x0[(t, 2)] = ("qk", qk_items[qi]); qi += 1
                slot_x0[(t, 5)] = ("qk", qk_items[qi]); qi += 1

            def run_slot(item):
                kind, arg = item
                if kind == "v":
                    emit_v_tile(arg)
                else:
                    emit_qk_tile(arg)

            # ================= Phase B: attention =================
            # Flat software-pipelined stream over steps s = 16t + 2jc + x.
            # PV for step s-PV_LAG is emitted at step s, so the in-order PE
            # queue never blocks on the exp->mul chain producing pm.
            pms = {}     # step -> [pm_ic0, pm_ic1]
            pv = {}      # t -> [x][ic] psum tiles
            eb = {}      # (t, jc) -> [128, 2048] tile (both heads)

            def emit_pv_half(s):
                t, r = divmod(s, 16)
                jc, x = divmod(r, 2)
                if t not in pv:
                    pv[t] = [[psum.tile([65, 512], f32, tag=f"pv{x_}{i}",
                                        name=f"pv{x_}{i}", bufs=1)
                              for i in range(2)] for x_ in range(2)]
                g = 2 * t + x
                for ic in range(2):
                    nc.tensor.matmul(
                        pv[t][x][ic][:],
                        v_sb[jc][:, g * 65:(g + 1) * 65],
                        pms[s][ic][:],
                        start=(jc == 0), stop=(jc == NT - 1),
                        skip_group_check=True,
                    )
                if jc == NT - 1 and x == 1:
                    emit_norm(t)

            def emit_norm(t):
                # row 64 of pv is the softmax denominator; reciprocal is
                # computed batched via a DRAM round trip (partition reshape +
                # stride-0 broadcast need DMA).
                ustage = work.tile([65, 2048], bf16, tag="ustage",
                                   name="ustage", bufs=2)
                for x in range(2):
                    for ic in range(2):
                        nc.vector.tensor_copy(
                            ustage[:, (2 * x + ic) * 512:(2 * x + ic + 1) * 512],
                            pv[t][x][ic][:])
                denom_d = dpool.tile([1, 2048], bf16, tag="denom_d", name="denom_d")
                nc.sync.dma_start(denom_d[:], ustage[64:65, :])
                dstage = work.tile([16, 128], bf16, tag="dstage", name="dstage", bufs=2)
                nc.sync.dma_start(
                    dstage[:],
                    denom_d[:].rearrange("a b -> (a b)").rearrange("(p c) -> p c", p=16))
                rstage = work.tile([16, 128], bf16, tag="rstage", name="rstage", bufs=2)
                with nc.allow_low_precision("softmax denom recip, 2e-2 gate"):
                    nc.vector.reciprocal(rstage[:], dstage[:])
                rd = dpool.tile([1, 2048], bf16, tag="rd", name="rd")
                nc.sync.dma_start(
                    rd[:].rearrange("a b -> (a b)").rearrange("(p c) -> p c", p=16),
                    rstage[:])
                rb = work.tile([64, 2048], bf16, tag="rb", name="rb", bufs=2)
                nc.sync.dma_start(
                    rb[:],
                    rd[:].rearrange("a b -> (a b)").unsqueeze(0).to_broadcast([64, 2048]))
                for x in range(2):
                    nc.vector.tensor_mul(
                        ot_sb[t][x * 64:(x + 1) * 64, :],
                        ustage[0:64, x * 1024:(x + 1) * 1024],
                        rb[:, x * 1024:(x + 1) * 1024],
                    )

            # phase-C partials: psum(ct 0..3) + pbias staged to SBUF in bf16
            # during t=5; a ct=4 pass runs after the PV flush (overlapping
            # the norm(5) DMA round trips) so the post-norm tail only runs
            # the ct=5 matmuls.
            cpart = [persist.tile([128, C], bf16, tag=f"cp{i}", name=f"cp{i}")
                     for i in range(NT)]

            def emit_cpart(nt, oc):
                ps = psum.tile([128, 512], f32, tag="big", name="pscp", bufs=4)
                for ct in range(4):
                    nc.tensor.matmul(
                        ps[:, 0:384],
                        ot_sb[ct][:, nt * 128:(nt + 1) * 128],
                        wproj_sb[:, ct * C + oc * 384:ct * C + (oc + 1) * 384],
                        start=(ct == 0), stop=(ct == 3),
                        skip_group_check=True,
                    )
                nc.vector.tensor_add(
                    cpart[nt][:, oc * 384:(oc + 1) * 384], ps[:, 0:384],
                    pb_bc[:, oc * 384:(oc + 1) * 384])

            def emit_cpart4(nt, oc):
                ps = psum.tile([128, 512], f32, tag="big", name="pscp4", bufs=4)
                nc.tensor.matmul(
                    ps[:, 0:384],
                    ot_sb[4][:, nt * 128:(nt + 1) * 128],
                    wproj_sb[:, 4 * C + oc * 384:4 * C + (oc + 1) * 384],
                    start=True, stop=True,
                    skip_group_check=True,
                )
                nc.vector.tensor_add(
                    cpart[nt][:, oc * 384:(oc + 1) * 384], ps[:, 0:384],
                    cpart[nt][:, oc * 384:(oc + 1) * 384])

            for s in range(96):
                t, r = divmod(s, 16)
                jc, x = divmod(r, 2)
                if t == 5 and x == 0:
                    emit_cpart(jc, 0)
                    emit_cpart(jc, 1)
                if x == 0:
                    ebt = work.tile([128, 2048], bf16, tag="eb", name="eb", bufs=5)
                    eb[(t, jc)] = ebt
                    nc.sync.dma_start(
                        ebt[:].rearrange("p (h n) -> p h n", h=2),
                        ebt_d.ap()[2 * t:2 * t + 2, jc * 128:(jc + 1) * 128, :]
                        .rearrange("h p n -> p h n"))
                    if (t, jc) in slot_x0:
                        run_slot(slot_x0[(t, jc)])
                else:
                    if (t, jc) in slot_x1:
                        run_slot(slot_x1[(t, jc)])
                if s == 64:
                    nc.sync.dma_start(
                        wproj_sb[:].rearrange("p (t c) -> p t c", t=CT),
                        wprojt_d.ap().rearrange("(t p) c -> p t c", p=128))
                    nc.sync.dma_start(
                        pb_bc[:], pbias_d.ap().unsqueeze(0).to_broadcast([128, C]))
                pms[s] = []
                for ic in range(2):
                    qs = psum.tile([128, 512], f32, tag="big", name="qkps", bufs=4)
                    nc.tensor.matmul(
                        qs[:],
                        k_pad[t][:, x * N + jc * 128:x * N + (jc + 1) * 128],
                        q_sb[t][:, ic * 512:(ic + 1) * 512],
                        start=True, stop=True,
                        skip_group_check=True,
                    )
                    pe = work.tile([128, 512], bf16, tag="pe", name="pe", bufs=8)
                    nc.scalar.activation(pe[:], qs[:], AF.Exp)
                    pm = work.tile([128, 512], bf16, tag="pm", name="pm", bufs=20)
                    nc.vector.tensor_mul(
                        pm[:], pe[:],
                        eb[(t, jc)][:, x * 1024 + ic * 512:x * 1024 + (ic + 1) * 512])
                    pms[s].append(pm)
                if s >= PV_LAG:
                    emit_pv_half(s - PV_LAG)
            for s in range(96 - PV_LAG, 96):
                emit_pv_half(s)
            for nt in range(NT):
                emit_cpart4(nt, 0)
                emit_cpart4(nt, 1)

            # ================= Phase C tail: ct 5 + partials ===============
            for nt in range(NT):
                pss = []
                for oc in range(2):
                    ps = psum.tile([128, 512], f32, tag="big", name="psc", bufs=4)
                    pss.append(ps)
                    nc.tensor.matmul(
                        ps[:, 0:384],
                        ot_sb[5][:, nt * 128:(nt + 1) * 128],
                        wproj_sb[:, 5 * C + oc * 384:5 * C + (oc + 1) * 384],
                        start=True, stop=True,
                        skip_group_check=True,
                    )
                osb = work.tile([128, C], bf16, tag="osb", name="osb", bufs=3)
                for oc in range(2):
                    nc.vector.tensor_add(
                        osb[:, oc * 384:(oc + 1) * 384], pss[oc][:, 0:384],
                        cpart[nt][:, oc * 384:(oc + 1) * 384])
                nc.sync.dma_start(out_d.ap()[nt * 128:(nt + 1) * 128, :], osb[:])

    nc.compile()
    return nc


def _get_nc():
    if "nc" not in _cache:
        _install_axon_shim()
        _cache["nc"] = build_nc()
    return _cache["nc"]


def prep_inputs(x, relative_position_index, qkv_weight, q_bias, v_bias,
                proj_weight, proj_bias, rel_pos_bias_table):
    """Host-side layout prep shared by all cores + per-core shards."""
    x = np.asarray(x, np.float32)
    idx = np.asarray(relative_position_index)
    qkv_weight = np.asarray(qkv_weight, np.float32)
    q_bias = np.asarray(q_bias, np.float32)
    v_bias = np.asarray(v_bias, np.float32)
    proj_weight = np.asarray(proj_weight, np.float32)
    proj_bias = np.asarray(proj_bias, np.float32)
    tbl = np.asarray(rel_pos_bias_table, np.float32)

    scale = (C // H) ** (-0.5)
    wq = qkv_weight.copy()
    wq[:C, :] *= scale  # fold softmax scale into q projection
    wqkvt = np.ascontiguousarray(wq.T).astype(BF16)  # [C, 3C]
    qkvb = np.concatenate([q_bias * scale, np.zeros_like(q_bias), v_bias]
                          ).astype(np.float32)
    wprojt = np.ascontiguousarray(proj_weight.T).astype(BF16)  # [C, C]

    # exp(bias) gather: ebt[h, j, i] = exp(table[idx[i, j], h])
    eb = np.exp(tbl)[idx]           # [i, j, H] f32
    ebt = np.ascontiguousarray(eb.transpose(2, 1, 0)).astype(BF16)  # [H, Nj, Ni]

    shared = {
        "wqkvt": wqkvt,
        "qkvb": qkvb,
        "vb": v_bias.astype(np.float32),
        "wprojt": wprojt,
        "pbias": proj_bias.astype(np.float32),
        "ebt": ebt,
    }
    in_maps = []
    for b in range(B):
        m = dict(shared)
        m["xt"] = np.ascontiguousarray(x[b].T).astype(BF16)  # [C, N]
        in_maps.append(m)
    return in_maps


def kernel(**inputs):
    from concourse.bass_utils import run_bass_kernel_spmd

    nc = _get_nc()
    in_maps = prep_inputs(**inputs)
    res = run_bass_kernel_spmd(nc, in_maps, list(range(N_CORES)),
                               trace=False)
    _cache["last_result"] = res
    out = np.stack([res.results[b]["out"] for b in range(B)], axis=0)
    return out.astype(np.float32)


def kernel_profiled(**inputs):
    """Same as kernel() but with NTFF tracing; returns (out, BassKernelResults)."""
    from concourse.bass_utils import run_bass_kernel_spmd

    nc = _get_nc()
    in_maps = prep_inputs(**inputs)
    res = run_bass_kernel_spmd(nc, in_maps, list(range(N_CORES)), trace=True)
    out = np.stack([res.results[b]["out"] for b in range(B)], axis=0)
    return out.astype(np.float32), res


# revision 60
# speedup vs baseline: 1.0002x; 1.0002x over previous
"""Trainium2 Bass kernel: multi-head attention with relative-position bias.

Problem shapes: x [8, 1024, 768], H=12 heads, d=64.
Strategy: data-parallel over batch (1 element per NeuronCore, 8 cores).
All matmuls in bf16 (f32 PSUM accumulation). Host prep:
  - weights transposed to [C, *] feature-major; q-scale folded into Wq/q_bias
  - relative-position bias gather done as exp(table)[idx] -> bf16 tensor
    [H, Nj, Ni] streamed from HBM and folded into softmax multiplicatively:
    softmax(s + b) = norm(exp(s) * exp(b))   (no row-max needed: |s| < ~10)
Attention computed transposed (sT[j, i]) so softmax sums run along the PE
contraction: the PV matmul uses stationary [v | 1], giving the denominator as
an extra psum row for free.

v2 restructure (from perfetto trace of v1 @246us):
  - warm-up matmuls at t=0 so the PE HAM clock-gate opens before real work
  - xt/wqkv DMAs issued first (sync DIRECT2D issue is ~0.6us each and was
    serializing startup); wproj/pbias deferred past phase-B emission
  - eb bias tiles batched 2-heads-per-DMA (half the DIRECT2D issues)
  - phase A (qkv projection) almost fully interleaved into phase B's
    scalar-engine-bound stretches: only q0/k0 tiles emitted up front
  - qs PSUM granularity [128,512]x4 banks with per-half exp/mul for finer
    ACT/PE pipelining (kills head-pair boundary stalls -> fewer HAM
    re-throttles)
  - normalization DMA dance batched: one ustage tile per head pair, single
    denominator row DMA [1,2048], one broadcast back [64,2048]
"""
import sys
import numpy as np

sys.path.insert(0, "/opt/trn_rl_repo")

import ml_dtypes

BF16 = ml_dtypes.bfloat16

B, N, C = 8, 1024, 768
H, D = 12, 64
N_CORES = 8
NT = N // 128        # 8 token tiles
CT = C // 128        # 6 feature tiles
OT = 3 * C // 128    # 18 qkv output feature tiles
PV_LAG = 8           # in (t, jc, x) x-steps: 8 = 4 jc iterations

_cache = {}


def _install_axon_shim():
    """The image's antenv lacks axon_hooks; register the NTFF profile hook so
    run_bass_kernel_spmd(trace=True) works. Safe no-op outside axon."""
    import types

    if "antenv.axon_hooks" not in sys.modules:
        try:
            import antenv
            from trn_agent_boot.trn_boot import _ntff_profile_via_ctypes
        except ImportError:
            return
        mod = types.ModuleType("antenv.axon_hooks")
        _hook = [None]
        mod.set_axon_ntff_profile_hook = lambda h: _hook.__setitem__(0, h)
        mod.get_axon_ntff_profile_hook = lambda: _hook[0]
        sys.modules["antenv.axon_hooks"] = mod
        antenv.axon_hooks = mod
        try:
            mod.set_axon_ntff_profile_hook(
                _ntff_profile_via_ctypes("/opt/axon/libaxon_pjrt.so")
            )
        except Exception:
            pass
    from concourse import bass_utils

    bass_utils.upload_artifacts = lambda tmpdir: tmpdir


def build_nc():
    from concourse import bacc, mybir, tile

    f32 = mybir.dt.float32
    bf16 = mybir.dt.bfloat16
    AF = mybir.ActivationFunctionType

    nc = bacc.Bacc("TRN2", target_bir_lowering=False, debug=False,
                   num_devices=N_CORES)

    xt_d = nc.dram_tensor("xt", [C, N], bf16, kind="ExternalInput")
    wqkvt_d = nc.dram_tensor("wqkvt", [C, 3 * C], bf16, kind="ExternalInput")
    qkvb_d = nc.dram_tensor("qkvb", [3 * C], f32, kind="ExternalInput")
    vb_d = nc.dram_tensor("vb", [C], f32, kind="ExternalInput")
    wprojt_d = nc.dram_tensor("wprojt", [C, C], bf16, kind="ExternalInput")
    pbias_d = nc.dram_tensor("pbias", [C], f32, kind="ExternalInput")
    ebt_d = nc.dram_tensor("ebt", [H, N, N], bf16, kind="ExternalInput")
    out_d = nc.dram_tensor("out", [N, C], bf16, kind="ExternalOutput")

    with tile.TileContext(nc) as tc:
        with (
            tc.tile_pool(name="persist", bufs=1) as persist,
            tc.tile_pool(name="work", bufs=1) as work,
            tc.tile_pool(name="dram", bufs=2, space="DRAM") as dpool,
            tc.tile_pool(name="psum", bufs=1, space="PSUM") as psum,
        ):
            # ---- resident tiles ----
            # q feature-major [head_pair][128, N]
            q_sb = [persist.tile([128, N], bf16, tag=f"q{i}", name=f"q{i}") for i in range(6)]
            # k zero-padded per head: [head_pair][128, 2N]; cols 0:N hold head
            # A's k in partitions 0:64 (64:128 zero), cols N:2N hold head B's
            # k in partitions 64:128 (0:64 zero).  K=128 stationaries keep
            # every matmul in 128x128 tiling mode: a 64-row stationary flips
            # the PE into 2x-row-tiled mode and each mode switch costs a
            # ~110ns pipeline drain on the next matmul.
            k_pad = [persist.tile([128, 2 * N], bf16, tag=f"kp{i}", name=f"kp{i}") for i in range(6)]
            # v token-major, 12 groups of (64 vals + 1 one) per token tile
            v_sb = [persist.tile([128, H * 65], bf16, tag=f"v{i}", name=f"v{i}") for i in range(NT)]
            # attention output (pre-proj), feature-major
            ot_sb = [persist.tile([128, N], bf16, tag=f"ot{i}", name=f"ot{i}") for i in range(CT)]

            # ---- PE warm-up: open the HAM clock gate while input DMAs run.
            warm = work.tile([128, 512], bf16, tag="warm")
            nc.vector.memset(warm[:], 0.0)
            for i in range(22):
                wps = psum.tile([128, 512], f32, tag="big", name="wps", bufs=4)
                nc.tensor.matmul(wps[:], warm[:, 0:128], warm[:],
                                 start=True, stop=True, skip_group_check=True)
            # k_pad zero regions (once; after the warm-up emission so the
            # warm-tile memset stays first in the vector-engine queue)
            for t in range(6):
                nc.vector.memset(k_pad[t][64:128, 0:N], 0.0)
                nc.vector.memset(k_pad[t][0:64, N:2 * N], 0.0)

            # ---- input DMAs: x and qkv weights first (per-ct so phase A can
            # start on the first pair), then small constants.
            xt_sb = [work.tile([128, N], bf16, tag=f"xt{i}", name=f"xt{i}") for i in range(CT)]
            wqkv_sb = [work.tile([128, 3 * C], bf16, tag=f"wq{i}", name=f"wq{i}") for i in range(CT)]
            for ct in range(CT):
                nc.sync.dma_start(xt_sb[ct][:], xt_d.ap()[ct * 128:(ct + 1) * 128, :])
                nc.sync.dma_start(wqkv_sb[ct][:], wqkvt_d.ap()[ct * 128:(ct + 1) * 128, :])

            qkvb_sb = persist.tile([128, OT], f32, tag="qkvb")
            vb_bc = persist.tile([128, C], f32, tag="vb_bc")
            nc.sync.dma_start(qkvb_sb[:], qkvb_d.ap().rearrange("(t p) -> p t", p=128))
            nc.sync.dma_start(vb_bc[:], vb_d.ap().unsqueeze(0).to_broadcast([128, C]))

            # wproj/pbias are deferred: emitted mid phase-B (only needed by C)
            wproj_sb = persist.tile([128, CT * C], bf16, tag="wp", name="wp")
            pb_bc = persist.tile([128, C], f32, tag="pb_bc")

            # ================= Phase A pieces =================
            # q,k feature-major: qkvT[o, n] = sum_c WT[c, o] * xT[c, n]
            def emit_qk_tile(ot):
                for h2 in range(2):
                    ps = psum.tile([128, 512], f32, tag="big", name="psa", bufs=4)
                    for ct in range(CT):
                        nc.tensor.matmul(
                            ps[:],
                            wqkv_sb[ct][:, ot * 128:(ot + 1) * 128],
                            xt_sb[ct][:, h2 * 512:(h2 + 1) * 512],
                            start=(ct == 0), stop=(ct == CT - 1),
                            skip_group_check=True,
                        )
                    if ot < 6:
                        nc.vector.tensor_scalar_add(
                            q_sb[ot][:, h2 * 512:(h2 + 1) * 512], ps[:],
                            qkvb_sb[:, ot:ot + 1])
                    else:
                        # k bias is zero; cast psum halves into padded layout
                        t = ot - 6
                        nc.vector.tensor_copy(
                            k_pad[t][0:64, h2 * 512:(h2 + 1) * 512],
                            ps[0:64, :])
                        nc.vector.tensor_copy(
                            k_pad[t][64:128, N + h2 * 512:N + (h2 + 1) * 512],
                            ps[64:128, :])

            # v token-major: v[n, vd] = sum_c xT[c, n] * WT[c, 2C+vd]
            # NOTE: a matmul output may not cross a 2KB PSUM bank, so the
            # 384-wide outputs go to separate [128,512] bank tiles.
            def emit_v_tile(nt):
                pss = []
                for g2 in range(2):
                    ps = psum.tile([128, 512], f32, tag="big", name="psv", bufs=4)
                    pss.append(ps)
                    for ct in range(CT):
                        nc.tensor.matmul(
                            ps[:, 0:384],
                            xt_sb[ct][:, nt * 128:(nt + 1) * 128],
                            wqkv_sb[ct][:, 2 * C + g2 * 384:2 * C + (g2 + 1) * 384],
                            start=(ct == 0), stop=(ct == CT - 1),
                            skip_group_check=True,
                        )
                v_view = v_sb[nt][:].rearrange("p (g c) -> p g c", c=65)
                for g2 in range(2):
                    nc.vector.tensor_add(
                        v_view[:, 6 * g2:6 * (g2 + 1), 0:64],
                        pss[g2][:, 0:384].rearrange("p (h c) -> p h c", c=64),
                        vb_bc[:, g2 * 384:(g2 + 1) * 384]
                        .rearrange("p (h c) -> p h c", c=64),
                    )
                nc.vector.memset(v_view[:, :, 64:65], 1.0)

            emit_qk_tile(0)
            emit_qk_tile(6)

            # deferred phase-A work, drained into phase B's slots
            # x1-slots (after head x=1 of a jc): v tiles in t=0
            # x0-slots: qk tiles, 2 per t starting at t=0 jc 6/7
            slot_x1 = {(0, jc): ("v", jc) for jc in range(NT)}
            slot_x0 = {}
            qk_items = []
            for t in range(1, 6):
                qk_items.append(t)
                qk_items.append(6 + t)
            slot_x0[(0, 5)] = ("qk", qk_items[0])
            slot_x0[(0, 6)] = ("qk", qk_items[1])
            qi = 2
            for t in range(1, 5):
                slot_x0[(t, 2)] = ("qk", qk_items[qi]); qi += 1
                slot_x0[(t, 5)] = ("qk", qk_items[qi]); qi += 1

            def run_slot(item):
                kind, arg = item
                if kind == "v":
                    emit_v_tile(arg)
                else:
                    emit_qk_tile(arg)

            # ================= Phase B: attention =================
            # Flat software-pipelined stream over steps s = 16t + 2jc + x.
            # PV for step s-PV_LAG is emitted at step s, so the in-order PE
            # queue never blocks on the exp->mul chain producing pm.
            pms = {}     # step -> [pm_ic0, pm_ic1]
            pv = {}      # t -> [x][ic] psum tiles
            eb = {}      # (t, jc) -> [128, 2048] tile (both heads)

            def emit_pv_half(s):
                t, r = divmod(s, 16)
                jc, x = divmod(r, 2)
                if t not in pv:
                    pv[t] = [[psum.tile([65, 512], f32, tag=f"pv{x_}{i}",
                                        name=f"pv{x_}{i}", bufs=1)
                              for i in range(2)] for x_ in range(2)]
                g = 2 * t + x
                for ic in range(2):
                    nc.tensor.matmul(
                        pv[t][x][ic][:],
                        v_sb[jc][:, g * 65:(g + 1) * 65],
                        pms[s][:, ic * 512:(ic + 1) * 512],
                        start=(jc == 0), stop=(jc == NT - 1),
                        skip_group_check=True,
                    )
                if jc == NT - 1 and x == 1:
                    emit_norm(t)

            def emit_norm(t):
                # row 64 of pv is the softmax denominator; reciprocal is
                # computed batched via a DRAM round trip (partition reshape +
                # stride-0 broadcast need DMA).
                # evacuate head A's pv banks on the scalar engine and head
                # B's on the vector engine so the two bank pairs drain in
                # parallel at the head-pair boundary
                ustage = work.tile([65, 2048], bf16, tag="ustage",
                                   name="ustage", bufs=2)
                for x in range(2):
                    for ic in range(2):
                        dst = ustage[:, (2 * x + ic) * 512:(2 * x + ic + 1) * 512]
                        if x == 0:
                            nc.scalar.activation(dst, pv[t][x][ic][:], AF.Copy)
                        else:
                            nc.vector.tensor_copy(dst, pv[t][x][ic][:])
                denom_d = dpool.tile([1, 2048], bf16, tag="denom_d", name="denom_d")
                nc.sync.dma_start(denom_d[:], ustage[64:65, :])
                dstage = work.tile([16, 128], bf16, tag="dstage", name="dstage", bufs=2)
                nc.sync.dma_start(
                    dstage[:],
                    denom_d[:].rearrange("a b -> (a b)").rearrange("(p c) -> p c", p=16))
                rstage = work.tile([16, 128], bf16, tag="rstage", name="rstage", bufs=2)
                with nc.allow_low_precision("softmax denom recip, 2e-2 gate"):
                    nc.vector.reciprocal(rstage[:], dstage[:])
                rd = dpool.tile([1, 2048], bf16, tag="rd", name="rd")
                nc.sync.dma_start(
                    rd[:].rearrange("a b -> (a b)").rearrange("(p c) -> p c", p=16),
                    rstage[:])
                rb = work.tile([64, 2048], bf16, tag="rb", name="rb", bufs=2)
                nc.sync.dma_start(
                    rb[:],
                    rd[:].rearrange("a b -> (a b)").unsqueeze(0).to_broadcast([64, 2048]))
                for x in range(2):
                    nc.vector.tensor_mul(
                        ot_sb[t][x * 64:(x + 1) * 64, :],
                        ustage[0:64, x * 1024:(x + 1) * 1024],
                        rb[:, x * 1024:(x + 1) * 1024],
                    )

            # phase-C partials: psum(ct 0..3) + pbias staged to SBUF in bf16
            # during t=5; a ct=4 pass runs after the PV flush (overlapping
            # the norm(5) DMA round trips) so the post-norm tail only runs
            # the ct=5 matmuls.
            cpart = [persist.tile([128, C], bf16, tag=f"cp{i}", name=f"cp{i}")
                     for i in range(NT)]

            def emit_cpart(nt, oc):
                ps = psum.tile([128, 512], f32, tag="big", name="pscp", bufs=4)
                for ct in range(4):
                    nc.tensor.matmul(
                        ps[:, 0:384],
                        ot_sb[ct][:, nt * 128:(nt + 1) * 128],
                        wproj_sb[:, ct * C + oc * 384:ct * C + (oc + 1) * 384],
                        start=(ct == 0), stop=(ct == 3),
                        skip_group_check=True,
                    )
                nc.vector.tensor_add(
                    cpart[nt][:, oc * 384:(oc + 1) * 384], ps[:, 0:384],
                    pb_bc[:, oc * 384:(oc + 1) * 384])

            def emit_cpart4(nt, oc):
                ps = psum.tile([128, 512], f32, tag="big", name="pscp4", bufs=4)
                nc.tensor.matmul(
                    ps[:, 0:384],
                    ot_sb[4][:, nt * 128:(nt + 1) * 128],
                    wproj_sb[:, 4 * C + oc * 384:4 * C + (oc + 1) * 384],
                    start=True, stop=True,
                    skip_group_check=True,
                )
                nc.vector.tensor_add(
                    cpart[nt][:, oc * 384:(oc + 1) * 384], ps[:, 0:384],
                    cpart[nt][:, oc * 384:(oc + 1) * 384])

            for s in range(96):
                t, r = divmod(s, 16)
                jc, x = divmod(r, 2)
                if t == 5 and x == 0:
                    emit_cpart(jc, 0)
                    emit_cpart(jc, 1)
                if x == 0:
                    ebt = work.tile([128, 2048], bf16, tag="eb", name="eb", bufs=5)
                    eb[(t, jc)] = ebt
                    nc.sync.dma_start(
                        ebt[:].rearrange("p (h n) -> p h n", h=2),
                        ebt_d.ap()[2 * t:2 * t + 2, jc * 128:(jc + 1) * 128, :]
                        .rearrange("h p n -> p h n"))
                    if (t, jc) in slot_x0:
                        run_slot(slot_x0[(t, jc)])
                else:
                    if (t, jc) in slot_x1:
                        run_slot(slot_x1[(t, jc)])
                if s == 64:
                    nc.sync.dma_start(
                        wproj_sb[:].rearrange("p (t c) -> p t c", t=CT),
                        wprojt_d.ap().rearrange("(t p) c -> p t c", p=128))
                    nc.sync.dma_start(
                        pb_bc[:], pbias_d.ap().unsqueeze(0).to_broadcast([128, C]))
                # exp stays per-[128,512]-half (one psum bank each) but both
                # halves land in one pe tile so the eb multiply runs as a
                # single full-width vector op (halves the DVE per-op
                # overhead); the later pm availability is absorbed by PV_LAG
                pe = work.tile([128, 1024], bf16, tag="pe", name="pe", bufs=4)
                for ic in range(2):
                    qs = psum.tile([128, 512], f32, tag="big", name="qkps", bufs=4)
                    nc.tensor.matmul(
                        qs[:],
                        k_pad[t][:, x * N + jc * 128:x * N + (jc + 1) * 128],
                        q_sb[t][:, ic * 512:(ic + 1) * 512],
                        start=True, stop=True,
                        skip_group_check=True,
                    )
                    nc.scalar.activation(pe[:, ic * 512:(ic + 1) * 512], qs[:],
                                         AF.Exp)
                pm = work.tile([128, 1024], bf16, tag="pm", name="pm", bufs=10)
                nc.vector.tensor_mul(
                    pm[:], pe[:], eb[(t, jc)][:, x * 1024:(x + 1) * 1024])
                pms[s] = pm
                if s >= PV_LAG:
                    emit_pv_half(s - PV_LAG)
            for s in range(96 - PV_LAG, 96):
                emit_pv_half(s)
            for nt in range(NT):
                emit_cpart4(nt, 0)
                emit_cpart4(nt, 1)

            # ================= Phase C tail: ct 5 + partials ===============
            for nt in range(NT):
                pss = []
                for oc in range(2):
                    ps = psum.tile([128, 512], f32, tag="big", name="psc", bufs=4)
                    pss.append(ps)
                    nc.tensor.matmul(
                        ps[:, 0:384],
                        ot_sb[5][:, nt * 128:(nt + 1) * 128],
                        wproj_sb[:, 5 * C + oc * 384:5 * C + (oc + 1) * 384],
                        start=True, stop=True,
                        skip_group_check=True,
                    )
                osb = work.tile([128, C], bf16, tag="osb", name="osb", bufs=3)
                for oc in range(2):
                    nc.vector.tensor_add(
                        osb[:, oc * 384:(oc + 1) * 384], pss[oc][:, 0:384],
                        cpart[nt][:, oc * 384:(oc + 1) * 384])
                nc.sync.dma_start(out_d.ap()[nt * 128:(nt + 1) * 128, :], osb[:])

    nc.compile()
    return nc


def _get_nc():
    if "nc" not in _cache:
        _install_axon_shim()
        _cache["nc"] = build_nc()
    return _cache["nc"]


def prep_inputs(x, relative_position_index, qkv_weight, q_bias, v_bias,
                proj_weight, proj_bias, rel_pos_bias_table):
    """Host-side layout prep shared by all cores + per-core shards."""
    x = np.asarray(x, np.float32)
    idx = np.asarray(relative_position_index)
    qkv_weight = np.asarray(qkv_weight, np.float32)
    q_bias = np.asarray(q_bias, np.float32)
    v_bias = np.asarray(v_bias, np.float32)
    proj_weight = np.asarray(proj_weight, np.float32)
    proj_bias = np.asarray(proj_bias, np.float32)
    tbl = np.asarray(rel_pos_bias_table, np.float32)

    scale = (C // H) ** (-0.5)
    wq = qkv_weight.copy()
    wq[:C, :] *= scale  # fold softmax scale into q projection
    wqkvt = np.ascontiguousarray(wq.T).astype(BF16)  # [C, 3C]
    qkvb = np.concatenate([q_bias * scale, np.zeros_like(q_bias), v_bias]
                          ).astype(np.float32)
    wprojt = np.ascontiguousarray(proj_weight.T).astype(BF16)  # [C, C]

    # exp(bias) gather: ebt[h, j, i] = exp(table[idx[i, j], h])
    eb = np.exp(tbl)[idx]           # [i, j, H] f32
    ebt = np.ascontiguousarray(eb.transpose(2, 1, 0)).astype(BF16)  # [H, Nj, Ni]

    shared = {
        "wqkvt": wqkvt,
        "qkvb": qkvb,
        "vb": v_bias.astype(np.float32),
        "wprojt": wprojt,
        "pbias": proj_bias.astype(np.float32),
        "ebt": ebt,
    }
    in_maps = []
    for b in range(B):
        m = dict(shared)
        m["xt"] = np.ascontiguousarray(x[b].T).astype(BF16)  # [C, N]
        in_maps.append(m)
    return in_maps


def kernel(**inputs):
    from concourse.bass_utils import run_bass_kernel_spmd

    nc = _get_nc()
    in_maps = prep_inputs(**inputs)
    res = run_bass_kernel_spmd(nc, in_maps, list(range(N_CORES)),
                               trace=False)
    _cache["last_result"] = res
    out = np.stack([res.results[b]["out"] for b in range(B)], axis=0)
    return out.astype(np.float32)


def kernel_profiled(**inputs):
    """Same as kernel() but with NTFF tracing; returns (out, BassKernelResults)."""
    from concourse.bass_utils import run_bass_kernel_spmd

    nc = _get_nc()
    in_maps = prep_inputs(**inputs)
    res = run_bass_kernel_spmd(nc, in_maps, list(range(N_CORES)), trace=True)
    out = np.stack([res.results[b]["out"] for b in range(B)], axis=0)
    return out.astype(np.float32), res


# revision 61
# speedup vs baseline: 1.0178x; 1.0176x over previous
"""Trainium2 Bass kernel: multi-head attention with relative-position bias.

Problem shapes: x [8, 1024, 768], H=12 heads, d=64.
Strategy: data-parallel over batch (1 element per NeuronCore, 8 cores).
All matmuls in bf16 (f32 PSUM accumulation). Host prep:
  - weights transposed to [C, *] feature-major; q-scale folded into Wq/q_bias
  - relative-position bias gather done as exp(table)[idx] -> bf16 tensor
    [H, Nj, Ni] streamed from HBM and folded into softmax multiplicatively:
    softmax(s + b) = norm(exp(s) * exp(b))   (no row-max needed: |s| < ~10)
Attention computed transposed (sT[j, i]) so softmax sums run along the PE
contraction: the PV matmul uses stationary [v | 1], giving the denominator as
an extra psum row for free.

v2 restructure (from perfetto trace of v1 @246us):
  - warm-up matmuls at t=0 so the PE HAM clock-gate opens before real work
  - xt/wqkv DMAs issued first (sync DIRECT2D issue is ~0.6us each and was
    serializing startup); wproj/pbias deferred past phase-B emission
  - eb bias tiles batched 2-heads-per-DMA (half the DIRECT2D issues)
  - phase A (qkv projection) almost fully interleaved into phase B's
    scalar-engine-bound stretches: only q0/k0 tiles emitted up front
  - qs PSUM granularity [128,512]x4 banks with per-half exp/mul for finer
    ACT/PE pipelining (kills head-pair boundary stalls -> fewer HAM
    re-throttles)
  - normalization DMA dance batched: one ustage tile per head pair, single
    denominator row DMA [1,2048], one broadcast back [64,2048]
"""
import sys
import numpy as np

sys.path.insert(0, "/opt/trn_rl_repo")

import ml_dtypes

BF16 = ml_dtypes.bfloat16

B, N, C = 8, 1024, 768
H, D = 12, 64
N_CORES = 8
NT = N // 128        # 8 token tiles
CT = C // 128        # 6 feature tiles
OT = 3 * C // 128    # 18 qkv output feature tiles
PV_LAG = 8           # in (t, jc, x) x-steps: 8 = 4 jc iterations

_cache = {}


def _install_axon_shim():
    """The image's antenv lacks axon_hooks; register the NTFF profile hook so
    run_bass_kernel_spmd(trace=True) works. Safe no-op outside axon."""
    import types

    if "antenv.axon_hooks" not in sys.modules:
        try:
            import antenv
            from trn_agent_boot.trn_boot import _ntff_profile_via_ctypes
        except ImportError:
            return
        mod = types.ModuleType("antenv.axon_hooks")
        _hook = [None]
        mod.set_axon_ntff_profile_hook = lambda h: _hook.__setitem__(0, h)
        mod.get_axon_ntff_profile_hook = lambda: _hook[0]
        sys.modules["antenv.axon_hooks"] = mod
        antenv.axon_hooks = mod
        try:
            mod.set_axon_ntff_profile_hook(
                _ntff_profile_via_ctypes("/opt/axon/libaxon_pjrt.so")
            )
        except Exception:
            pass
    from concourse import bass_utils

    bass_utils.upload_artifacts = lambda tmpdir: tmpdir


def build_nc():
    from concourse import bacc, mybir, tile

    f32 = mybir.dt.float32
    bf16 = mybir.dt.bfloat16
    AF = mybir.ActivationFunctionType

    nc = bacc.Bacc("TRN2", target_bir_lowering=False, debug=False,
                   num_devices=N_CORES)

    xt_d = nc.dram_tensor("xt", [C, N], bf16, kind="ExternalInput")
    wqkvt_d = nc.dram_tensor("wqkvt", [C, 3 * C], bf16, kind="ExternalInput")
    qkvb_d = nc.dram_tensor("qkvb", [3 * C], f32, kind="ExternalInput")
    vb_d = nc.dram_tensor("vb", [C], f32, kind="ExternalInput")
    wprojt_d = nc.dram_tensor("wprojt", [C, C], bf16, kind="ExternalInput")
    pbias_d = nc.dram_tensor("pbias", [C], f32, kind="ExternalInput")
    ebt_d = nc.dram_tensor("ebt", [H, N, N], bf16, kind="ExternalInput")
    out_d = nc.dram_tensor("out", [N, C], bf16, kind="ExternalOutput")

    with tile.TileContext(nc) as tc:
        with (
            tc.tile_pool(name="persist", bufs=1) as persist,
            tc.tile_pool(name="work", bufs=1) as work,
            tc.tile_pool(name="dram", bufs=2, space="DRAM") as dpool,
            tc.tile_pool(name="psum", bufs=1, space="PSUM") as psum,
        ):
            # ---- resident tiles ----
            # q feature-major [head_pair][128, N]
            q_sb = [persist.tile([128, N], bf16, tag=f"q{i}", name=f"q{i}") for i in range(6)]
            # k zero-padded per head: [head_pair][128, 2N]; cols 0:N hold head
            # A's k in partitions 0:64 (64:128 zero), cols N:2N hold head B's
            # k in partitions 64:128 (0:64 zero).  K=128 stationaries keep
            # every matmul in 128x128 tiling mode: a 64-row stationary flips
            # the PE into 2x-row-tiled mode and each mode switch costs a
            # ~110ns pipeline drain on the next matmul.
            k_pad = [persist.tile([128, 2 * N], bf16, tag=f"kp{i}", name=f"kp{i}") for i in range(6)]
            # v token-major, 12 groups of (64 vals + 1 one) per token tile
            v_sb = [persist.tile([128, H * 65], bf16, tag=f"v{i}", name=f"v{i}") for i in range(NT)]
            # attention output (pre-proj), feature-major
            ot_sb = [persist.tile([128, N], bf16, tag=f"ot{i}", name=f"ot{i}") for i in range(CT)]

            # ---- PE warm-up: open the HAM clock gate while input DMAs run.
            warm = work.tile([128, 512], bf16, tag="warm")
            nc.vector.memset(warm[:], 0.0)
            for i in range(22):
                wps = psum.tile([128, 512], f32, tag="big", name="wps", bufs=4)
                nc.tensor.matmul(wps[:], warm[:, 0:128], warm[:],
                                 start=True, stop=True, skip_group_check=True)
            # k_pad zero regions (once; after the warm-up emission so the
            # warm-tile memset stays first in the vector-engine queue)
            for t in range(6):
                nc.vector.memset(k_pad[t][64:128, 0:N], 0.0)
                nc.vector.memset(k_pad[t][0:64, N:2 * N], 0.0)

            # ---- input DMAs: x and qkv weights first (per-ct so phase A can
            # start on the first pair), then small constants.
            xt_sb = [work.tile([128, N], bf16, tag=f"xt{i}", name=f"xt{i}") for i in range(CT)]
            wqkv_sb = [work.tile([128, 3 * C], bf16, tag=f"wq{i}", name=f"wq{i}") for i in range(CT)]
            for ct in range(CT):
                nc.sync.dma_start(xt_sb[ct][:], xt_d.ap()[ct * 128:(ct + 1) * 128, :])
                nc.sync.dma_start(wqkv_sb[ct][:], wqkvt_d.ap()[ct * 128:(ct + 1) * 128, :])

            qkvb_sb = persist.tile([128, OT], f32, tag="qkvb")
            vb_bc = persist.tile([128, C], f32, tag="vb_bc")
            nc.sync.dma_start(qkvb_sb[:], qkvb_d.ap().rearrange("(t p) -> p t", p=128))
            nc.sync.dma_start(vb_bc[:], vb_d.ap().unsqueeze(0).to_broadcast([128, C]))

            # wproj/pbias are deferred: emitted mid phase-B (only needed by C)
            wproj_sb = persist.tile([128, CT * C], bf16, tag="wp", name="wp")
            pb_bc = persist.tile([128, C], f32, tag="pb_bc")

            # ================= Phase A pieces =================
            # q,k feature-major: qkvT[o, n] = sum_c WT[c, o] * xT[c, n]
            def emit_qk_tile(ot):
                for h2 in range(2):
                    ps = psum.tile([128, 512], f32, tag="big", name="psa", bufs=4)
                    for ct in range(CT):
                        nc.tensor.matmul(
                            ps[:],
                            wqkv_sb[ct][:, ot * 128:(ot + 1) * 128],
                            xt_sb[ct][:, h2 * 512:(h2 + 1) * 512],
                            start=(ct == 0), stop=(ct == CT - 1),
                            skip_group_check=True,
                        )
                    if ot < 6:
                        nc.vector.tensor_scalar_add(
                            q_sb[ot][:, h2 * 512:(h2 + 1) * 512], ps[:],
                            qkvb_sb[:, ot:ot + 1])
                    else:
                        # k bias is zero; cast psum halves into padded layout
                        t = ot - 6
                        nc.vector.tensor_copy(
                            k_pad[t][0:64, h2 * 512:(h2 + 1) * 512],
                            ps[0:64, :])
                        nc.vector.tensor_copy(
                            k_pad[t][64:128, N + h2 * 512:N + (h2 + 1) * 512],
                            ps[64:128, :])

            # v token-major: v[n, vd] = sum_c xT[c, n] * WT[c, 2C+vd]
            # NOTE: a matmul output may not cross a 2KB PSUM bank, so the
            # 384-wide outputs go to separate [128,512] bank tiles.
            def emit_v_tile(nt):
                pss = []
                for g2 in range(2):
                    ps = psum.tile([128, 512], f32, tag="big", name="psv", bufs=4)
                    pss.append(ps)
                    for ct in range(CT):
                        nc.tensor.matmul(
                            ps[:, 0:384],
                            xt_sb[ct][:, nt * 128:(nt + 1) * 128],
                            wqkv_sb[ct][:, 2 * C + g2 * 384:2 * C + (g2 + 1) * 384],
                            start=(ct == 0), stop=(ct == CT - 1),
                            skip_group_check=True,
                        )
                v_view = v_sb[nt][:].rearrange("p (g c) -> p g c", c=65)
                for g2 in range(2):
                    nc.vector.tensor_add(
                        v_view[:, 6 * g2:6 * (g2 + 1), 0:64],
                        pss[g2][:, 0:384].rearrange("p (h c) -> p h c", c=64),
                        vb_bc[:, g2 * 384:(g2 + 1) * 384]
                        .rearrange("p (h c) -> p h c", c=64),
                    )
                nc.vector.memset(v_view[:, :, 64:65], 1.0)

            emit_qk_tile(0)
            emit_qk_tile(6)

            # deferred phase-A work, drained into phase B's slots
            # x1-slots (after head x=1 of a jc): v tiles in t=0
            # x0-slots: qk tiles, 2 per t starting at t=0 jc 6/7
            slot_x1 = {(0, jc): ("v", jc) for jc in range(NT)}
            slot_x0 = {}
            qk_items = []
            for t in range(1, 6):
                qk_items.append(t)
                qk_items.append(6 + t)
            slot_x0[(0, 5)] = ("qk", qk_items[0])
            slot_x0[(0, 6)] = ("qk", qk_items[1])
            qi = 2
            for t in range(1, 5):
                slot_x0[(t, 2)] = ("qk", qk_items[qi]); qi += 1
                slot_x0[(t, 5)] = ("qk", qk_items[qi]); qi += 1

            def run_slot(item):
                kind, arg = item
                if kind == "v":
                    emit_v_tile(arg)
                else:
                    emit_qk_tile(arg)

            # ================= Phase B: attention =================
            # Flat software-pipelined stream over steps s = 16t + 2jc + x.
            # PV for step s-PV_LAG is emitted at step s, so the in-order PE
            # queue never blocks on the exp->mul chain producing pm.
            pms = {}     # step -> [pm_ic0, pm_ic1]
            pv = {}      # t -> [x][ic] psum tiles
            eb = {}      # (t, jc) -> [128, 2048] tile (both heads)

            def emit_pv_half(s):
                t, r = divmod(s, 16)
                jc, x = divmod(r, 2)
                if t not in pv:
                    pv[t] = [[psum.tile([65, 512], f32, tag=f"pv{x_}{i}",
                                        name=f"pv{x_}{i}", bufs=1)
                              for i in range(2)] for x_ in range(2)]
                g = 2 * t + x
                for ic in range(2):
                    nc.tensor.matmul(
                        pv[t][x][ic][:],
                        v_sb[jc][:, g * 65:(g + 1) * 65],
                        pms[s][ic][:],
                        start=(jc == 0), stop=(jc == NT - 1),
                        skip_group_check=True,
                    )
                if jc == NT - 1 and x == 1:
                    emit_norm(t)

            def emit_norm(t):
                # row 64 of pv is the softmax denominator; reciprocal is
                # computed batched via a DRAM round trip (partition reshape +
                # stride-0 broadcast need DMA).
                ustage = work.tile([65, 2048], bf16, tag="ustage",
                                   name="ustage", bufs=2)
                for x in range(2):
                    for ic in range(2):
                        nc.vector.tensor_copy(
                            ustage[:, (2 * x + ic) * 512:(2 * x + ic + 1) * 512],
                            pv[t][x][ic][:])
                denom_d = dpool.tile([1, 2048], bf16, tag="denom_d", name="denom_d")
                nc.sync.dma_start(denom_d[:], ustage[64:65, :])
                dstage = work.tile([16, 128], bf16, tag="dstage", name="dstage", bufs=2)
                nc.sync.dma_start(
                    dstage[:],
                    denom_d[:].rearrange("a b -> (a b)").rearrange("(p c) -> p c", p=16))
                rstage = work.tile([16, 128], bf16, tag="rstage", name="rstage", bufs=2)
                with nc.allow_low_precision("softmax denom recip, 2e-2 gate"):
                    nc.vector.reciprocal(rstage[:], dstage[:])
                rd = dpool.tile([1, 2048], bf16, tag="rd", name="rd")
                nc.sync.dma_start(
                    rd[:].rearrange("a b -> (a b)").rearrange("(p c) -> p c", p=16),
                    rstage[:])
                rb = work.tile([64, 2048], bf16, tag="rb", name="rb", bufs=2)
                nc.sync.dma_start(
                    rb[:],
                    rd[:].rearrange("a b -> (a b)").unsqueeze(0).to_broadcast([64, 2048]))
                for x in range(2):
                    nc.vector.tensor_mul(
                        ot_sb[t][x * 64:(x + 1) * 64, :],
                        ustage[0:64, x * 1024:(x + 1) * 1024],
                        rb[:, x * 1024:(x + 1) * 1024],
                    )

            # phase-C partials: psum(ct 0..3) + pbias staged to SBUF in bf16
            # during t=5; a ct=4 pass runs after the PV flush (overlapping
            # the norm(5) DMA round trips) so the post-norm tail only runs
            # the ct=5 matmuls.
            cpart = [persist.tile([128, C], bf16, tag=f"cp{i}", name=f"cp{i}")
                     for i in range(NT)]

            def emit_cpart(nt, oc):
                ps = psum.tile([128, 512], f32, tag="big", name="pscp", bufs=4)
                for ct in range(4):
                    nc.tensor.matmul(
                        ps[:, 0:384],
                        ot_sb[ct][:, nt * 128:(nt + 1) * 128],
                        wproj_sb[:, ct * C + oc * 384:ct * C + (oc + 1) * 384],
                        start=(ct == 0), stop=(ct == 3),
                        skip_group_check=True,
                    )
                nc.vector.tensor_add(
                    cpart[nt][:, oc * 384:(oc + 1) * 384], ps[:, 0:384],
                    pb_bc[:, oc * 384:(oc + 1) * 384])

            def emit_cpart4(nt, oc):
                ps = psum.tile([128, 512], f32, tag="big", name="pscp4", bufs=4)
                nc.tensor.matmul(
                    ps[:, 0:384],
                    ot_sb[4][:, nt * 128:(nt + 1) * 128],
                    wproj_sb[:, 4 * C + oc * 384:4 * C + (oc + 1) * 384],
                    start=True, stop=True,
                    skip_group_check=True,
                )
                nc.vector.tensor_add(
                    cpart[nt][:, oc * 384:(oc + 1) * 384], ps[:, 0:384],
                    cpart[nt][:, oc * 384:(oc + 1) * 384])

            for s in range(96):
                t, r = divmod(s, 16)
                jc, x = divmod(r, 2)
                if t == 5 and x == 0:
                    emit_cpart(jc, 0)
                    emit_cpart(jc, 1)
                if x == 0:
                    ebt = work.tile([128, 2048], bf16, tag="eb", name="eb", bufs=5)
                    eb[(t, jc)] = ebt
                    nc.sync.dma_start(
                        ebt[:].rearrange("p (h n) -> p h n", h=2),
                        ebt_d.ap()[2 * t:2 * t + 2, jc * 128:(jc + 1) * 128, :]
                        .rearrange("h p n -> p h n"))
                    if (t, jc) in slot_x0:
                        run_slot(slot_x0[(t, jc)])
                else:
                    if (t, jc) in slot_x1:
                        run_slot(slot_x1[(t, jc)])
                if s == 64:
                    nc.sync.dma_start(
                        wproj_sb[:].rearrange("p (t c) -> p t c", t=CT),
                        wprojt_d.ap().rearrange("(t p) c -> p t c", p=128))
                    nc.sync.dma_start(
                        pb_bc[:], pbias_d.ap().unsqueeze(0).to_broadcast([128, C]))
                pms[s] = []
                for ic in range(2):
                    qs = psum.tile([128, 512], f32, tag="big", name="qkps", bufs=4)
                    nc.tensor.matmul(
                        qs[:],
                        k_pad[t][:, x * N + jc * 128:x * N + (jc + 1) * 128],
                        q_sb[t][:, ic * 512:(ic + 1) * 512],
                        start=True, stop=True,
                        skip_group_check=True,
                    )
                    pe = work.tile([128, 512], bf16, tag="pe", name="pe", bufs=8)
                    nc.scalar.activation(pe[:], qs[:], AF.Exp)
                    pm = work.tile([128, 512], bf16, tag="pm", name="pm", bufs=20)
                    nc.vector.tensor_mul(
                        pm[:], pe[:],
                        eb[(t, jc)][:, x * 1024 + ic * 512:x * 1024 + (ic + 1) * 512])
                    pms[s].append(pm)
                if s >= PV_LAG:
                    emit_pv_half(s - PV_LAG)
            for s in range(96 - PV_LAG, 96):
                emit_pv_half(s)
            for nt in range(NT):
                emit_cpart4(nt, 0)
                emit_cpart4(nt, 1)

            # ================= Phase C tail: ct 5 + partials ===============
            for nt in range(NT):
                pss = []
                for oc in range(2):
                    ps = psum.tile([128, 512], f32, tag="big", name="psc", bufs=4)
                    pss.append(ps)
                    nc.tensor.matmul(
                        ps[:, 0:384],
                        ot_sb[5][:, nt * 128:(nt + 1) * 128],
                        wproj_sb[:, 5 * C + oc * 384:5 * C + (oc + 1) * 384],
                        start=True, stop=True,
                        skip_group_check=True,
                    )
                osb = work.tile([128, C], bf16, tag="osb", name="osb", bufs=3)
                for oc in range(2):
                    nc.vector.tensor_add(
                        osb[:, oc * 384:(oc + 1) * 384], pss[oc][:, 0:384],
                        cpart[nt][:, oc * 384:(oc + 1) * 384])
                nc.sync.dma_start(out_d.ap()[nt * 128:(nt + 1) * 128, :], osb[:])

    nc.compile()
    return nc


def _get_nc():
    if "nc" not in _cache:
        _install_axon_shim()
        _cache["nc"] = build_nc()
    return _cache["nc"]


def prep_inputs(x, relative_position_index, qkv_weight, q_bias, v_bias,
                proj_weight, proj_bias, rel_pos_bias_table):
    """Host-side layout prep shared by all cores + per-core shards."""
    x = np.asarray(x, np.float32)
    idx = np.asarray(relative_position_index)
    qkv_weight = np.asarray(qkv_weight, np.float32)
    q_bias = np.asarray(q_bias, np.float32)
    v_bias = np.asarray(v_bias, np.float32)
    proj_weight = np.asarray(proj_weight, np.float32)
    proj_bias = np.asarray(proj_bias, np.float32)
    tbl = np.asarray(rel_pos_bias_table, np.float32)

    scale = (C // H) ** (-0.5)
    wq = qkv_weight.copy()
    wq[:C, :] *= scale  # fold softmax scale into q projection
    wqkvt = np.ascontiguousarray(wq.T).astype(BF16)  # [C, 3C]
    qkvb = np.concatenate([q_bias * scale, np.zeros_like(q_bias), v_bias]
                          ).astype(np.float32)
    wprojt = np.ascontiguousarray(proj_weight.T).astype(BF16)  # [C, C]

    # exp(bias) gather: ebt[h, j, i] = exp(table[idx[i, j], h])
    eb = np.exp(tbl)[idx]           # [i, j, H] f32
    ebt = np.ascontiguousarray(eb.transpose(2, 1, 0)).astype(BF16)  # [H, Nj, Ni]

    shared = {
        "wqkvt": wqkvt,
        "qkvb": qkvb,
        "vb": v_bias.astype(np.float32),
        "wprojt": wprojt,
        "pbias": proj_bias.astype(np.float32),
        "ebt": ebt,
    }
    in_maps = []
    for b in range(B):
        m = dict(shared)
        m["xt"] = np.ascontiguousarray(x[b].T).astype(BF16)  # [C, N]
        in_maps.append(m)
    return in_maps


def kernel(**inputs):
    from concourse.bass_utils import run_bass_kernel_spmd

    nc = _get_nc()
    in_maps = prep_inputs(**inputs)
    res = run_bass_kernel_spmd(nc, in_maps, list(range(N_CORES)),
                               trace=False)
    _cache["last_result"] = res
    out = np.stack([res.results[b]["out"] for b in range(B)], axis=0)
    return out.astype(np.float32)


def kernel_profiled(**inputs):
    """Same as kernel() but with NTFF tracing; returns (out, BassKernelResults)."""
    from concourse.bass_utils import run_bass_kernel_spmd

    nc = _get_nc()
    in_maps = prep_inputs(**inputs)
    res = run_bass_kernel_spmd(nc, in_maps, list(range(N_CORES)), trace=True)
    out = np.stack([res.results[b]["out"] for b in range(B)], axis=0)
    return out.astype(np.float32), res
